# revision 28
# baseline (speedup 1.0000x reference)
"""Trainium2 Bass kernel for nn_CombinedLoss (sinkhorn-KD + soft-CE + embed MSE).

v2 architecture (8 cores):
  - All inputs shipped bf16. Logits in q-major [B, QS, T] per-core layout so a
    single XBAR DMA-transpose per tensor yields [q, t, b] gram operands
    (no PE transposes, no PSUM evacuation of transposed data).
  - Grams per pair: [G_xx | G_xy] and G_yy only; G_yx derived on the owner
    core as G_xy^T after reduction.
  - Two bf16 AllReduces in [mat*128+b, j] block layout: C1 (pairs 0/1 grams +
    CE/a/embed/diag cols) fires after pair 1 and hides under pair-2 compute;
    C2 (pair-2 grams + pc2/diag2) is the only exposed collective.
  - Phase B (9 sinkhorn iterations) sharded: each core processes only 2 of
    the 12 B x B matrices, selected via per-core indirect-DMA row gathers and
    data-driven update rules (uniform SPMD program).
  - Final loss_kd partials stay per-core; the host sums the 8 scalar outputs
    (the unshard step - all loss reductions are sums over shards).
"""
import os
import numpy as np

B = 128
T = 50
Q = 1024
S = 49          # MAX_STEP - 1
H = 256
NCORES = 8
QS = Q // NCORES
TEMP = 0.5
GSCALE = 1.0 / (TEMP * TEMP)   # p-gram = GSCALE * logit-gram = 4
RHO = 500.0 ** 2
EPS_FINAL = 0.005 ** 2
SUP_W, DIST_W, EMBED_W, LOSS_WEIGHT = 1.0, 0.01, 1.0, 1.0
CKD = float(LOSS_WEIGHT * DIST_W * (RHO + EPS_FINAL / 2.0) / B)

# embed t-shard split (padded to 7 per core)
ESPLIT = [7, 7, 6, 6, 6, 6, 6, 6]
EOFF = [0, 7, 14, 20, 26, 32, 38, 44]
EPAD = 7

# c1 AllReduce buffer: [1024, 128] bf16 rows; mat k occupies rows 128k..128k+127
# mats: xy0=0, xy1=1, xx0=2, xx1=3, yy0=4, yy1=5; CE-extra blocks rows 768-1023
# CE-extra cols (within the logical [B, 256] block):
#   pc0 at 0-63, pc1 64-127, a 128-191, embed 192, diag01 193-196
C1ROWS = 1024
# c2: xy2=0, xx2=1, yy2=2; CE2-extra rows 384-511: pc2 0-63, diag2 64-65
C2ROWS = 512

C1MAT = {('xy', 0): 0, ('xy', 1): 1, ('xx', 0): 2, ('xx', 1): 3,
         ('yy', 0): 4, ('yy', 1): 5}
C2MAT = {('xy', 2): 0, ('xx', 2): 1, ('yy', 2): 2}

# per-core slot assignment: (slot0 src, slot1 src); None = derived/dead
# a0=1: slot0 <- c1 read; a0=0: slot0 <- transpose(c2 read). etc.
SLOTS = {
    0: dict(i0=('xy', 0), i1=None, a0=1, a1=0, pf=1,
            kc=[-CKD, -CKD], rs=[('x', 0), ('y', 0)], cs=[('y', 0), ('x', 0)]),
    1: dict(i0=('xy', 1), i1=None, a0=1, a1=0, pf=1,
            kc=[-CKD, -CKD], rs=[('x', 1), ('y', 1)], cs=[('y', 1), ('x', 1)]),
    2: dict(i0=None, i1=('xy', 2), a0=0, a1=1, pf=1,
            kc=[-CKD, -CKD], rs=[('y', 2), ('x', 2)], cs=[('x', 2), ('y', 2)]),
    3: dict(i0=('xx', 0), i1=('xy', 2), a0=1, a1=1, pf=0,
            kc=[CKD, 0.0], rs=[('x', 0), ('x', 2)], cs=[('x', 0), ('y', 2)]),
    4: dict(i0=('xx', 1), i1=('xy', 2), a0=1, a1=1, pf=0,
            kc=[CKD, 0.0], rs=[('x', 1), ('x', 2)], cs=[('x', 1), ('y', 2)]),
    5: dict(i0=('xy', 0), i1=('xx', 2), a0=1, a1=1, pf=0,
            kc=[0.0, CKD], rs=[('x', 0), ('x', 2)], cs=[('y', 0), ('x', 2)]),
    6: dict(i0=('yy', 0), i1=('yy', 2), a0=1, a1=1, pf=0,
            kc=[CKD, CKD], rs=[('y', 0), ('y', 2)], cs=[('y', 0), ('y', 2)]),
    7: dict(i0=('yy', 1), i1=('xy', 2), a0=1, a1=1, pf=0,
            kc=[CKD, 0.0], rs=[('y', 1), ('y', 2)], cs=[('y', 1), ('y', 2)]),
}
# diag6 order: [dxx0, dyy0, dxx1, dyy1, dxx2, dyy2]
DIDX = {('x', 0): 0, ('y', 0): 1, ('x', 1): 2, ('y', 1): 3,
        ('x', 2): 4, ('y', 2): 5}


def _eps_schedule():
    eps_list = []
    e = 1.0
    while e > EPS_FINAL:
        eps_list.append(e)
        e = e * 0.25
    eps_list.append(EPS_FINAL)
    return eps_list


def build_bass():
    import concourse.bass as bass
    import concourse.bacc as bacc
    import concourse.tile as tile
    from concourse import mybir
    from concourse.masks import make_identity

    f32 = mybir.dt.float32
    bf16 = mybir.dt.bfloat16
    i32 = mybir.dt.int32
    Alu = mybir.AluOpType
    Act = mybir.ActivationFunctionType
    X = mybir.AxisListType.X

    nc = bacc.Bacc(
        "TRN2",
        target_bir_lowering=False,
        debug=False,
        num_devices=NCORES,
    )

    xs = [nc.declare_dram_parameter(n, [QS, T, B], bf16, isOutput=False)
          for n in ("xc", "xt", "xe")]
    ys = [nc.declare_dram_parameter(n, [QS, T, B], bf16, isOutput=False)
          for n in ("yc", "yt", "ye")]
    xst = [nc.declare_dram_parameter(n, [B, S, QS], bf16, isOutput=False)
           for n in ("xct", "xtt", "xet")]
    dbc = nc.declare_dram_parameter("dbc", [B, S, QS], bf16, isOutput=False)
    dbn = nc.declare_dram_parameter("dbn", [B, S, QS], bf16, isOutput=False)
    ehs = nc.declare_dram_parameter("ehs", [B, EPAD, H], bf16, isOutput=False)
    eht = nc.declare_dram_parameter("eht", [B, EPAD, H], bf16, isOutput=False)
    eds = nc.declare_dram_parameter("eds", [B, EPAD, H], bf16, isOutput=False)
    edt = nc.declare_dram_parameter("edt", [B, EPAD, H], bf16, isOutput=False)
    auxf = nc.declare_dram_parameter("auxf", [B, 14], f32, isOutput=False)
    auxb = nc.declare_dram_parameter("auxb", [B, 28], bf16, isOutput=False)
    auxi = nc.declare_dram_parameter("auxi", [B, 2], i32, isOutput=False)
    out_ext = nc.declare_dram_parameter("out", [1, 1], f32, isOutput=True)

    c1_in = nc.dram_tensor("c1_in", [C1ROWS, 128], bf16)
    c1_out = nc.dram_tensor("c1_out", [C1ROWS, 128], bf16, addr_space="Shared")
    c2_in = nc.dram_tensor("c2_in", [C2ROWS, 128], bf16)
    c2_out = nc.dram_tensor("c2_out", [C2ROWS, 128], bf16, addr_space="Shared")

    # constants baked into the NEFF
    import ml_dtypes
    msk_np = np.zeros((2, 256), np.float32)
    msk_np[0, 0:128] = 1.0
    msk_np[1, 128:256] = 1.0
    msk_dram = nc.inline_tensor(msk_np.astype(ml_dtypes.bfloat16), "mskc")
    idx_np = np.broadcast_to(np.arange(64, dtype=np.float32), (B, 64)).copy()
    idx_dram = nc.inline_tensor(idx_np, "idxc")

    blog = float(-np.log(float(B)))
    groups = [list(range(NCORES))]

    with nc.allow_low_precision(reason="bf16 partial sums are exact or slack-validated"), \
         tile.TileContext(nc) as tc:
        with tc.tile_pool(name="persist", bufs=1) as persist:
            identf = persist.tile([128, 128], f32)
            make_identity(nc, identf[:])
            identb = persist.tile([128, 128], bf16)
            nc.vector.tensor_copy(identb[:], identf[:])

            cesb = persist.tile([B, 256], bf16)
            nc.vector.memset(cesb[:], 0.0)
            cesb2 = persist.tile([B, 128], bf16)
            nc.vector.memset(cesb2[:], 0.0)
            delta = persist.tile([B, S, QS], bf16)

            # ---------------- phase A ----------------
            with (
                tc.tile_pool(name="sload", bufs=2) as sload,
                tc.tile_pool(name="bload", bufs=1) as bload,
                tc.tile_pool(name="tpool", bufs=3) as tpool,
                tc.tile_pool(name="mpool", bufs=2) as mpool,
                tc.tile_pool(name="epool", bufs=1) as epool,
                tc.tile_pool(name="ecomp", bufs=1) as ecomp,
                tc.tile_pool(name="gevac", bufs=2) as gevac,
                tc.tile_pool(name="gpsum", bufs=2, space="PSUM") as gpsum,
            ):
                # issue every input DMA up front (in-order queues; nothing here
                # depends on compute, so the queues stream at full rate)
                # batch + CE inputs go on the (otherwise idle) gpsimd SWDGE
                # queue so the DVE work can start while the gram loads stream
                # on the two HWDGE queues
                bct = bload.tile([B, S, QS], bf16, tag="bc")
                nc.gpsimd.dma_start(out=bct[:], in_=dbc[:, :, :])
                bnt = bload.tile([B, S, QS], bf16, tag="bn")
                nc.gpsimd.dma_start(out=bnt[:], in_=dbn[:, :, :])
                xtms = []
                for p in range(3):
                    xtm = sload.tile([B, S, QS], bf16, tag="xs")
                    nc.gpsimd.dma_start(out=xtm[:], in_=xst[p][:, :, :])
                    xtms.append(xtm)
                tps = []
                for p in range(3):
                    tp = tpool.tile([128, 2, T, 128], bf16, tag="tp")
                    nc.sync.dma_start(out=tp[:, 0, :, :], in_=xs[p][:, :, :])
                    nc.scalar.dma_start(out=tp[:, 1, :, :], in_=ys[p][:, :, :])
                    tps.append(tp)
                epairs = []
                for k, (ea, eb) in enumerate(((ehs, eht), (eds, edt))):
                    e1 = epool.tile([B, EPAD * H], bf16, tag=f"ea{k}")
                    nc.scalar.dma_start(out=e1[:],
                                        in_=ea[:].rearrange("b t h -> b (t h)"))
                    e2 = epool.tile([B, EPAD * H], bf16, tag=f"eb{k}")
                    nc.sync.dma_start(out=e2[:],
                                        in_=eb[:].rearrange("b t h -> b (t h)"))
                    epairs.append((e1, e2))

                # batch: delta + a partials (t-major, contiguous reductions)
                nc.vector.tensor_add(delta[:], bct[:], bnt[:])
                dif = bload.tile([B, S, QS], bf16, tag="dif")
                nc.vector.tensor_sub(dif[:], bct[:], bnt[:])
                nc.vector.reduce_sum(out=cesb[:, 128:128 + S], in_=dif[:],
                                     axis=X)

                # grams + CE gathers per pair
                for p in range(3):
                    tp = tps[p]
                    gpa = gpsum.tile([128, 256], f32, tag="ga")
                    gyy = gpsum.tile([128, 128], f32, tag="gy")
                    for t in range(T):
                        nc.tensor.matmul(
                            gpa[:], tp[:, 0, t, :], tp[:, :, t, :],
                            start=(t == 0), stop=(t == T - 1))
                        nc.tensor.matmul(
                            gyy[:], tp[:, 1, t, :], tp[:, 1, t, :],
                            start=(t == 0), stop=(t == T - 1))

                    # CE gather for this (student) pair (t-major, contiguous)
                    ms = mpool.tile([B, S, QS], bf16, tag="ms")
                    nc.vector.tensor_mul(ms[:], xtms[p][:], delta[:])
                    pcdst = (cesb[:, 64 * p:64 * p + S] if p < 2
                             else cesb2[:, 0:S])
                    nc.vector.reduce_sum(out=pcdst, in_=ms[:], axis=X)

                    # evacuate grams (bf16) + diag partials
                    gsb = gevac.tile([B, 384], bf16, tag="gsb")
                    nc.scalar.copy(gsb[:, 0:256], gpa[:])
                    nc.vector.tensor_copy(gsb[:, 256:384], gyy[:])
                    dsc = mpool.tile([B, 128], bf16, tag="dsc")
                    ddst = (cesb[:, 193 + 2 * p:195 + 2 * p] if p < 2
                            else cesb2[:, 64:66])
                    nc.vector.tensor_mul(dsc[:], gsb[:, 0:128], identb[:])
                    nc.vector.reduce_sum(out=ddst[:, 0:1], in_=dsc[:], axis=X)
                    dsc2 = mpool.tile([B, 128], bf16, tag="dsc")
                    nc.vector.tensor_mul(dsc2[:], gsb[:, 256:384], identb[:])
                    nc.vector.reduce_sum(out=ddst[:, 1:2], in_=dsc2[:], axis=X)

                    # stage mat blocks to collective input buffers
                    if p < 2:
                        xyr, xxr, yyr = 128 * p, 128 * (2 + p), 128 * (4 + p)
                        nc.sync.dma_start(out=c1_in[xyr:xyr + 128, :],
                                          in_=gsb[:, 128:256])
                        nc.sync.dma_start(out=c1_in[xxr:xxr + 128, :],
                                          in_=gsb[:, 0:128])
                        nc.sync.dma_start(out=c1_in[yyr:yyr + 128, :],
                                          in_=gsb[:, 256:384])
                    else:
                        nc.sync.dma_start(out=c2_in[0:128, :],
                                          in_=gsb[:, 128:256])
                        nc.sync.dma_start(out=c2_in[128:256, :],
                                          in_=gsb[:, 0:128])
                        nc.sync.dma_start(out=c2_in[256:384, :],
                                          in_=gsb[:, 256:384])

                    if p == 1:
                        # CE-extra block rides C1; fire C1 while pair 2 runs
                        nc.sync.dma_start(out=c1_in[768:896, :],
                                          in_=cesb[:, 0:128])
                        nc.sync.dma_start(out=c1_in[896:1024, :],
                                          in_=cesb[:, 128:256])
                        nc.gpsimd.collective_compute(
                            "AllReduce", Alu.add, replica_groups=groups,
                            ins=[c1_in[:, :]], outs=[c1_out[:, :]])

                # embed partials (gpsimd sub keeps the DVE queue clear; the
                # result rides C2, which fires last anyway)
                ecols = persist.tile([B, 2], f32)
                for k, (e1, e2) in enumerate(epairs):
                    ed = ecomp.tile([B, EPAD * H], bf16, tag="ed")
                    nc.gpsimd.tensor_sub(ed[:], e1[:], e2[:])
                    esq = ecomp.tile([B, EPAD * H], bf16, tag="esq")
                    nc.scalar.activation(esq[:], ed[:], Act.Square,
                                         accum_out=ecols[:, k:k + 1])
                embf = persist.tile([B, 1], f32)
                nc.vector.tensor_add(embf[:], ecols[:, 0:1], ecols[:, 1:2])
                nc.vector.tensor_copy(cesb2[:, 66:67], embf[:])
                nc.sync.dma_start(out=c2_in[384:512, :], in_=cesb2[:])
                nc.gpsimd.collective_compute(
                    "AllReduce", Alu.add, replica_groups=groups,
                    ins=[c2_in[:, :]], outs=[c2_out[:, :]])

            # ---------------- phase B ----------------
            with (
                tc.tile_pool(name="pbig", bufs=2) as pbig,
                tc.tile_pool(name="psmall", bufs=2) as psmall,
                tc.tile_pool(name="pconst", bufs=1) as pconst,
                tc.tile_pool(name="hps", bufs=2, space="PSUM") as hpsum,
                tc.tile_pool(name="fps", bufs=2, space="PSUM") as fpsum,
                tc.tile_pool(name="sps", bufs=1, space="PSUM") as spsum,
            ):
                # aux inputs
                axf = pconst.tile([B, 14], f32)
                nc.scalar.dma_start(out=axf[:], in_=auxf[:, :])
                axb = pconst.tile([B, 28], bf16)
                nc.scalar.dma_start(out=axb[:], in_=auxb[:, :])
                axi = pconst.tile([B, 2], i32)
                nc.scalar.dma_start(out=axi[:], in_=auxi[:, :])
                mskt = pconst.tile([2, 256], bf16)
                nc.scalar.dma_start(out=mskt[:], in_=msk_dram[:, :])
                ones2b = pconst.tile([2, 128], bf16)
                nc.vector.memset(ones2b[:], 1.0)
                ones_col = pconst.tile([B, 1], f32)
                nc.vector.memset(ones_col[:], 1.0)

                # CE-extra replicated blocks
                c1post = pconst.tile([B, 256], bf16)
                nc.sync.dma_start(out=c1post[:, 0:128], in_=c1_out[768:896, :])
                nc.sync.dma_start(out=c1post[:, 128:256], in_=c1_out[896:1024, :])
                c2post = pconst.tile([B, 128], bf16)
                nc.sync.dma_start(out=c2post[:], in_=c2_out[384:512, :])

                # slot gathers (per-core indices)
                r0 = pconst.tile([B, 128], bf16)
                nc.gpsimd.indirect_dma_start(
                    out=r0[:], out_offset=None,
                    in_=c1_out[0:768, :],
                    in_offset=bass.IndirectOffsetOnAxis(ap=axi[:, 0:1], axis=0))
                r1 = pconst.tile([B, 128], bf16)
                nc.gpsimd.indirect_dma_start(
                    out=r1[:], out_offset=None,
                    in_=c2_out[0:384, :],
                    in_offset=bass.IndirectOffsetOnAxis(ap=axi[:, 1:2], axis=0))
                t0p = fpsum.tile([128, 128], bf16, tag="t0")
                nc.tensor.transpose(t0p[:], r0[:], identb[:])
                t0 = pconst.tile([B, 128], bf16)
                nc.vector.tensor_copy(t0[:], t0p[:])
                t1p = fpsum.tile([128, 128], bf16, tag="t0")
                nc.tensor.transpose(t1p[:], r1[:], identb[:])
                t1 = pconst.tile([B, 128], bf16)
                nc.vector.tensor_copy(t1[:], t1p[:])
                # Gsl[:,0,:] = a0*r0 + (1-a0)*t1 ; Gsl[:,1,:] = a1*r1 + (1-a1)*t0
                Gsl = pconst.tile([B, 2, 128], bf16)
                gtmp = psmall.tile([B, 128], bf16, tag="gtmp")
                nc.vector.tensor_scalar(gtmp[:], t1[:], axf[:, 11:12], None,
                                        Alu.mult)
                nc.vector.scalar_tensor_tensor(Gsl[:, 0, :], r0[:], axf[:, 10:11],
                                               gtmp[:], Alu.mult, Alu.add)
                gtmp2 = psmall.tile([B, 128], bf16, tag="gtmp")
                nc.vector.tensor_scalar(gtmp2[:], t0[:], axf[:, 13:14], None,
                                        Alu.mult)
                nc.vector.scalar_tensor_tensor(Gsl[:, 1, :], r1[:], axf[:, 12:13],
                                               gtmp2[:], Alu.mult, Alu.add)

                # diag6 + per-slot D2 / DH via shipped selection masks
                diag6 = pconst.tile([B, 6], bf16)
                nc.vector.tensor_copy(diag6[:, 0:4], c1post[:, 193:197])
                nc.vector.tensor_copy(diag6[:, 4:6], c2post[:, 64:66])
                D2 = pconst.tile([B, 2], f32)
                DH = pconst.tile([B, 2], f32)
                for s in range(2):
                    selr = axb[:, 4 + 6 * s:10 + 6 * s]
                    selc = axb[:, 16 + 6 * s:22 + 6 * s]
                    dt_ = psmall.tile([B, 6], bf16, tag="dt")
                    nc.vector.tensor_mul(dt_[:], diag6[:], selr)
                    nc.vector.reduce_sum(out=D2[:, s:s + 1], in_=dt_[:], axis=X)
                    dt2 = psmall.tile([B, 6], bf16, tag="dt")
                    nc.vector.tensor_mul(dt2[:], diag6[:], selc)
                    nc.vector.reduce_sum(out=DH[:, s:s + 1], in_=dt2[:], axis=X)

                F = pconst.tile([B, 2], f32)
                nc.vector.memset(F[:], 0.0)
                sv = pconst.tile([B, 2], f32)
                mv = pconst.tile([B, 2], f32)
                scrx = pconst.tile([B, 2, 128], bf16)
                escr = pconst.tile([B, 128], bf16)

                # ln on DVE: exponent/mantissa split + deg-2 poly, max err 6e-3
                # (plenty for eps*ln; keeps the scalar engine Exp-only ->
                # no act-table reloads)
                LN2 = 0.6931471805599453
                LC2, LC1, LC0 = (-0.23351351824407424, 1.3827825718019444,
                                 -1.1430148212645563)

                def dve_ln(dst, src, n):
                    svi = src.bitcast(i32)
                    sh = psmall.tile([B, n], i32, tag="lsh")
                    nc.vector.tensor_scalar(sh[:], svi, 23, None,
                                            Alu.logical_shift_right)
                    ef = psmall.tile([B, n], f32, tag="lef")
                    nc.vector.tensor_copy(ef[:], sh[:])
                    mi = psmall.tile([B, n], i32, tag="lmi")
                    nc.vector.tensor_scalar(mi[:], svi, 0x007FFFFF, 0x3F800000,
                                            Alu.bitwise_and, Alu.bitwise_or)
                    t1 = psmall.tile([B, n], f32, tag="lt1")
                    nc.vector.tensor_scalar(t1[:], mi[:].bitcast(f32), LC2, LC1,
                                            Alu.mult, Alu.add)
                    t2 = psmall.tile([B, n], f32, tag="lt2")
                    nc.vector.tensor_tensor(t2[:], t1[:], mi[:].bitcast(f32),
                                            Alu.mult)
                    e2f = psmall.tile([B, n], f32, tag="le2")
                    nc.vector.tensor_scalar(e2f[:], ef[:], LN2,
                                            -127.0 * LN2 + LC0,
                                            Alu.mult, Alu.add)
                    nc.vector.tensor_tensor(dst, e2f[:], t2[:], Alu.add)

                for eps in _eps_schedule():
                    damp = 1.0 / (1.0 + eps / RHO)
                    c = GSCALE / eps
                    fsum = psmall.tile([B, 2], f32, tag="fsum")
                    nc.vector.tensor_add(fsum[:], F[:], DH[:])
                    ftp = fpsum.tile([2, 128], f32, tag="ft")
                    nc.tensor.transpose(ftp[:], fsum[:], identf[:])
                    HT = psmall.tile([2, 128], bf16, tag="ht")
                    nc.vector.tensor_scalar(HT[:], ftp[:], 1.0 / GSCALE,
                                            blog * eps / GSCALE,
                                            Alu.mult, Alu.add)
                    rhm = psmall.tile([2, 2, 128], bf16, tag="rhm")
                    nc.vector.tensor_tensor(
                        rhm[:], HT[:].unsqueeze(1).broadcast_to((2, 2, 128)),
                        mskt[:].rearrange("k (a j) -> k a j", j=128), Alu.mult)
                    hbt = hpsum.tile([128, 256], f32, tag="hb")
                    nc.tensor.matmul(hbt[:], ones2b[:],
                                     rhm[:].rearrange("k a j -> k (a j)"),
                                     start=True, stop=False)
                    nc.tensor.matmul(hbt[:], identb[:],
                                     Gsl[:].rearrange("b a j -> b (a j)"),
                                     start=False, stop=True)
                    hb3 = hbt[:].rearrange("b (s j) -> b s j", j=128)
                    nc.vector.reduce_max(out=mv[:], in_=hb3, axis=X)
                    nc.vector.tensor_tensor(
                        scrx[:], hb3,
                        mv[:].unsqueeze(2).broadcast_to((B, 2, 128)),
                        Alu.subtract)
                    for s in range(2):
                        nc.scalar.activation(escr[:], scrx[:, s, :], Act.Exp,
                                             scale=float(c),
                                             accum_out=sv[:, s:s + 1])
                    lg = psmall.tile([B, 2], f32, tag="lg")
                    dve_ln(lg[:], sv[:], 2)
                    # dmu = D2 - eps*lg - GSCALE*mv
                    dm1 = psmall.tile([B, 2], f32, tag="dm1")
                    nc.vector.scalar_tensor_tensor(dm1[:], lg[:], float(-eps),
                                                   D2[:], Alu.mult, Alu.add)
                    dmu = psmall.tile([B, 2], f32, tag="dmu")
                    nc.vector.scalar_tensor_tensor(dmu[:], mv[:], float(-GSCALE),
                                                   dm1[:], Alu.mult, Alu.add)
                    dr = psmall.tile([B, 2], f32, tag="dr")
                    nc.vector.tensor_copy(dr[:, 0:1], dmu[:, 1:2])
                    nc.vector.tensor_copy(dr[:, 1:2], dmu[:, 0:1])
                    # cmix = damp * ((1-pf)*dmu + pf*rev(dmu))
                    c1t = psmall.tile([B, 2], f32, tag="c1t")
                    nc.vector.tensor_scalar(c1t[:], dmu[:], axf[:, 1:2],
                                            float(damp), Alu.mult, Alu.mult)
                    c2t = psmall.tile([B, 2], f32, tag="c2t")
                    nc.vector.tensor_scalar(c2t[:], dr[:], axf[:, 0:1],
                                            float(damp), Alu.mult, Alu.mult)
                    cmix = psmall.tile([B, 2], f32, tag="cmix")
                    nc.vector.tensor_add(cmix[:], c1t[:], c2t[:])
                    # F = wF*F + vF*cmix
                    m1 = psmall.tile([B, 2], f32, tag="m1")
                    nc.vector.tensor_mul(m1[:], F[:], axf[:, 2:4])
                    m2 = psmall.tile([B, 2], f32, tag="m2")
                    nc.vector.tensor_mul(m2[:], cmix[:], axf[:, 4:6])
                    nc.vector.tensor_add(F[:], m1[:], m2[:])

                # ---- loss_kd partial ----
                E2 = psmall.tile([B, 2], f32, tag="e2")
                nc.scalar.activation(E2[:], F[:], Act.Exp, scale=float(-1.0 / RHO))
                km = psmall.tile([B, 2], f32, tag="km")
                nc.vector.tensor_mul(km[:], E2[:], axf[:, 6:8])
                kdp = psmall.tile([B, 1], f32, tag="kdp")
                nc.vector.reduce_sum(out=kdp[:], in_=km[:], axis=X)

                # ---- CE (replicated; gated by aux csup/cemb) ----
                pcall = pconst.tile([B, 192], f32)
                nc.vector.tensor_copy(pcall[:, 0:128], c1post[:, 0:128])
                nc.vector.tensor_copy(pcall[:, 128:192], c2post[:, 0:64])
                af = pconst.tile([B, 64], f32)
                nc.vector.tensor_copy(af[:], c1post[:, 128:192])
                embcol = pconst.tile([B, 1], f32)
                nc.vector.tensor_copy(embcol[:], c2post[:, 66:67])

                idxf = pconst.tile([B, 64], f32)
                nc.scalar.dma_start(out=idxf[:], in_=idx_dram[:, :])
                pos = psmall.tile([B, 64], f32, tag="pos")
                nc.vector.tensor_scalar(pos[:], pcall[:, 0:64], 0.0, None,
                                        Alu.is_gt)
                ip1 = psmall.tile([B, 64], f32, tag="ip1")
                nc.vector.scalar_tensor_tensor(ip1[:], idxf[:], 1.0, pos[:],
                                               Alu.add, Alu.mult)
                Lp = psmall.tile([B, 1], f32, tag="Lp")
                nc.vector.reduce_max(out=Lp[:], in_=ip1[:], axis=X)
                eq0 = psmall.tile([B, 1], f32, tag="eq0")
                nc.vector.tensor_scalar(eq0[:], Lp[:], 0.0, None, Alu.is_equal)
                Lv = psmall.tile([B, 1], f32, tag="Lv")
                nc.vector.scalar_tensor_tensor(Lv[:], eq0[:], float(S), Lp[:],
                                               Alu.mult, Alu.add)
                dl = psmall.tile([B, 64], f32, tag="dl")
                nc.vector.tensor_scalar(dl[:], idxf[:], Lv[:, 0:1], None,
                                        Alu.subtract)
                mask = psmall.tile([B, 64], f32, tag="mask")
                nc.vector.tensor_scalar(mask[:], dl[:], 0.0, None, Alu.is_lt)
                negf = psmall.tile([B, 64], f32, tag="negf")
                nc.vector.tensor_scalar(negf[:], mask[:], 1.0, 1e9,
                                        Alu.subtract, Alu.mult)
                # a = floor((asum+1)/2) via magic round (values < 2^22)
                MAGIC = 12582912.0
                tv = psmall.tile([B, 64], f32, tag="tv")
                nc.vector.tensor_scalar(tv[:], af[:], 0.5, 1024.25,
                                        Alu.mult, Alu.add)
                tm = psmall.tile([B, 64], f32, tag="tm")
                nc.vector.tensor_scalar(tm[:], tv[:], MAGIC, MAGIC,
                                        Alu.add, Alu.subtract)
                av = psmall.tile([B, 64], f32, tag="av")
                nc.vector.tensor_scalar(av[:], tm[:], 1024.0, None, Alu.subtract)
                amask = psmall.tile([B, 64], f32, tag="amask")
                nc.vector.tensor_tensor(amask[:], av[:], mask[:], Alu.mult)
                pc3 = pcall[:].rearrange("b (s q) -> b s q", q=64)
                mce = pbig.tile([B, 3, 64], f32, tag="mce")
                mask3 = mask[:].unsqueeze(1).broadcast_to((B, 3, 64))
                negf3 = negf[:].unsqueeze(1).broadcast_to((B, 3, 64))
                amask3 = amask[:].unsqueeze(1).broadcast_to((B, 3, 64))
                t2_ = pbig.tile([B, 3, 64], f32, tag="tt")
                nc.vector.scalar_tensor_tensor(t2_[:], pc3, 2.0, mask3, Alu.mult,
                                               Alu.mult)
                nc.vector.tensor_tensor(mce[:], t2_[:], negf3, Alu.add)
                mx3 = psmall.tile([B, 3], f32, tag="mx3")
                nc.vector.reduce_max(out=mx3[:], in_=mce[:], axis=X)
                mb3 = mx3[:].unsqueeze(2).broadcast_to((B, 3, 64))
                dd = pbig.tile([B, 3, 64], f32, tag="dd")
                nc.vector.tensor_tensor(dd[:], mce[:], mb3, Alu.subtract)
                ee = pbig.tile([B, 3, 64], f32, tag="ee")
                nc.scalar.activation(ee[:], dd[:], Act.Exp)
                ss3 = psmall.tile([B, 3], f32, tag="ss3")
                nc.vector.reduce_sum(out=ss3[:], in_=ee[:], axis=X)
                lg3 = psmall.tile([B, 3], f32, tag="lg3")
                dve_ln(lg3[:], ss3[:], 3)
                lse3 = psmall.tile([B, 3], f32, tag="lse3")
                nc.vector.tensor_add(lse3[:], mx3[:], lg3[:])
                lb3 = lse3[:].unsqueeze(2).broadcast_to((B, 3, 64))
                d1 = pbig.tile([B, 3, 64], f32, tag="dd")
                nc.vector.tensor_tensor(d1[:], mce[:], lb3, Alu.subtract)
                d2_ = pbig.tile([B, 3, 64], f32, tag="tt")
                nc.vector.tensor_tensor(d2_[:], d1[:], amask3, Alu.mult)
                rowsum = psmall.tile([B, 1], f32, tag="rs")
                nc.vector.reduce_sum(out=rowsum[:],
                                     in_=d2_[:].rearrange("b s q -> b (s q)"),
                                     axis=X)

                # ---- final combine: csup*CE + cemb*embed + kd_partial ----
                tot_ps = spsum.tile([1, 1], f32, tag="tot")
                nc.tensor.matmul(tot_ps[:], rowsum[:], axf[:, 8:9], start=True,
                                 stop=False)
                nc.tensor.matmul(tot_ps[:], embcol[:], axf[:, 9:10], start=False,
                                 stop=False)
                nc.tensor.matmul(tot_ps[:], kdp[:], ones_col[:], start=False,
                                 stop=True)
                outt = psmall.tile([1, 1], f32, tag="outt")
                nc.vector.tensor_copy(outt[:], tot_ps[:])
                nc.sync.dma_start(out=out_ext[:, :], in_=outt[:])

    nc.compile()
    return nc


_NC = None
LAST_RESULTS = None


def _core_aux(c):
    sl = SLOTS[c]
    i0 = C1MAT[sl['i0']] if sl['i0'] is not None else 0
    i1 = C2MAT[sl['i1']] if sl['i1'] is not None else 0
    pf = float(sl['pf'])
    wf = [0.0, 0.0] if sl['pf'] else [0.5, 0.5]
    vf = [1.0, 1.0] if sl['pf'] else [0.5, 0.5]
    cgate = 1.0 if c == 0 else 0.0
    auxf = np.zeros((B, 14), np.float32)
    auxf[:, 0] = pf
    auxf[:, 1] = 1.0 - pf
    auxf[:, 2:4] = wf
    auxf[:, 4:6] = vf
    auxf[:, 6:8] = sl['kc']
    auxf[:, 8] = -LOSS_WEIGHT * SUP_W * cgate
    auxf[:, 9] = LOSS_WEIGHT * EMBED_W * 0.5 * cgate
    auxf[:, 10] = float(sl['a0'])
    auxf[:, 11] = 1.0 - float(sl['a0'])
    auxf[:, 12] = float(sl['a1'])
    auxf[:, 13] = 1.0 - float(sl['a1'])
    auxb = np.zeros((B, 28), np.float32)
    auxb[:, 0] = float(sl['a0'])
    auxb[:, 1] = 1.0 - float(sl['a0'])
    auxb[:, 2] = float(sl['a1'])
    auxb[:, 3] = 1.0 - float(sl['a1'])
    for s in range(2):
        auxb[:, 4 + 6 * s + DIDX[sl['rs'][s]]] = 2.0
        auxb[:, 16 + 6 * s + DIDX[sl['cs'][s]]] = -2.0
    auxi = np.zeros((B, 2), np.int32)
    auxi[:, 0] = 128 * i0 + np.arange(B)
    auxi[:, 1] = 128 * i1 + np.arange(B)
    return auxf, auxb, auxi


def _shard_inputs(logit_c, logit_t, logit_ensemble, logit_teacher_c,
                  logit_teacher_t, logit_teacher_ensemble, out_h_student,
                  out_h_teacher, out_d_student, out_d_teacher, batch):
    import ml_dtypes
    bf = ml_dtypes.bfloat16
    students = [logit_c, logit_t, logit_ensemble]
    teachers = [logit_teacher_c, logit_teacher_t, logit_teacher_ensemble]
    embeds = dict(ehs=out_h_student, eht=out_h_teacher,
                  eds=out_d_student, edt=out_d_teacher)
    # q-major [B, QS, T] bf16 per core (XBAR transpose source) + t-major
    # [B, S, QS] copies for the contiguous CE/delta path
    sbf = [np.asarray(a, np.float32).astype(bf) for a in students]
    sb = [np.ascontiguousarray(np.transpose(a, (0, 2, 1))) for a in sbf]
    tb = [np.ascontiguousarray(np.transpose(
        np.asarray(a, np.float32).astype(bf), (0, 2, 1))) for a in teachers]
    bct = np.asarray(batch[:, 1:1 + S, :Q], np.float32).astype(bf)
    bnt = np.asarray(batch[:, 1:1 + S, Q:], np.float32).astype(bf)
    in_maps = []
    for c in range(NCORES):
        q0 = QS * c
        m = {}
        for nm, arr in zip(("xc", "xt", "xe"), sb):
            m[nm] = np.ascontiguousarray(arr[:, q0:q0 + QS, :].transpose(1, 2, 0))
        for nm, arr in zip(("yc", "yt", "ye"), tb):
            m[nm] = np.ascontiguousarray(arr[:, q0:q0 + QS, :].transpose(1, 2, 0))
        for nm, arr in zip(("xct", "xtt", "xet"), sbf):
            m[nm] = np.ascontiguousarray(arr[:, 0:S, q0:q0 + QS])
        m["dbc"] = np.ascontiguousarray(bct[:, :, q0:q0 + QS])
        m["dbn"] = np.ascontiguousarray(bnt[:, :, q0:q0 + QS])
        t0, w = EOFF[c], ESPLIT[c]
        for nm, arr in embeds.items():
            sl = np.zeros((B, EPAD, H), bf)
            sl[:, :w, :] = np.asarray(arr[:, t0:t0 + w, :], np.float32).astype(bf)
            m[nm] = sl
        axf, axb, axi = _core_aux(c)
        m["auxf"] = axf
        m["auxb"] = axb.astype(bf)
        m["auxi"] = axi
        in_maps.append(m)
    return in_maps


def kernel(**inputs):
    global _NC, LAST_RESULTS
    from concourse.bass_utils import run_bass_kernel_spmd
    if _NC is None:
        _NC = build_bass()
    in_maps = _shard_inputs(**inputs)
    trace = bool(int(os.environ.get("KERNEL_TRACE", "0")))
    res = run_bass_kernel_spmd(_NC, in_maps, list(range(NCORES)), trace=trace)
    LAST_RESULTS = res
    total = sum(float(np.asarray(r["out"]).reshape(-1)[0]) for r in res.results)
    return np.asarray([total], dtype=np.float32)
